# revision 20
# baseline (speedup 1.0000x reference)
"""MoD router Trainium2 kernel.

Computes, for hidden_states [4, 4096, 2048] and gate_w [1, 2048]:
    scores = einsum("bsh,h->bs", hidden_states, gate_w[0])        # [4, 4096]
    mask   = top-k mask per batch row (k = 2048 = S/2), 1.0/0.0   # [4, 4096]
returns (mask, scores), matching the reference.

Distribution: the B*S = 16384 score rows are sharded 8 ways (2048 rows per
NeuronCore; cores 2b and 2b+1 cover the two halves of batch row b). Per core:
  1. Stream its [2048, 2048] f32 slab of hidden_states (16 MiB) in 2 MiB
     chunks; fused multiply+reduce against the (host-prebroadcast) gate
     vector on the vector engine -> 2048 scores.  The kernel is DMA-bound
     (A/B probe: removing all DVE matvec work changes nothing); 2 MiB
     chunks beat both 1 MiB and 4 MiB in same-REPS A/B tests.
  2. PE-transpose scores to flat row order, DMA out; pairwise AllGather
     (groups {2b, 2b+1}) so both cores of a pair hold the full 4096-score
     row (8 KB payload).
  3. Broadcast the row to all 128 partitions via PE select-matmul; evict
     PSUM->SBUF on the scalar engine.
  4. 3-level 127-ary threshold search, split DVE/ACT: each level compares
     the row against 128 candidate pivots (DVE is_ge count on the first
     S-N_ACT cols at 2x f32 mode, ACT Sign-sum count on the last N_ACT
     cols; sign ties are provably benign), then derives the next 128
     pivots with a closed-form linear update (PE ones-matmul for the
     cross-partition count + one ACT op; all level constants folded into
     host-precomputed d columns).  Final pivot vector = tau replicated.
  5. mask = (scores >= tau), DMA out.

The rep loop is software-pipelined with a 1-rep skew (emit_search of rep
k-1 inside iteration k) so no engine's in-order stream waits on the
current rep's collective before starting the next rep's bulk work.
Engine busy per body (steady state): DMA ~38 us (bound), DVE ~38 us,
ACT ~16 us, PE ~5 us, collective ~5 us.  Measured 2026-08-09: ~37.8 us
per body (vs 87.8 us baseline), mask exact, scores rel err 8.4e-7.

Timing-probe flags (production values: _MATVEC_COLS=H, _N_PE_TILES=0):
_MATVEC_COLS=1 keeps the DMA stream but strips DVE matvec work;
_N_PE_TILES>0 moves trailing row-tiles to a PE-transpose + ACT-evict +
accumulating-matmul path (measured useless: the kernel is DMA-bound).
"""

import numpy as np

B, S, H = 4, 4096, 2048
N_CORES = 8
R = (B * S) // N_CORES      # rows per core = 2048
RT = R // 128               # 128-row tiles per core = 16
K_TOP = S // 2              # 2048
NA = 2                      # row-tiles per DMA chunk (chunk = NA MiB)
N_LEVELS = 3
W0 = 2.0                    # initial bracket [-1, 1]
# pivot cell widths per level (c[l] = W0 / 127^(l+1)); c[N_LEVELS] = 0
CS = [W0 / (127.0 ** (l + 1)) for l in range(N_LEVELS)] + [0.0]
N_ACT = 3584                # cols counted on ACT (rest on DVE)
COMBINE_K = float(2 * K_TOP - N_ACT)

_CACHE = {}
_REPS = 1   # repeat whole body inside one NEFF (timing aid)
_MATVEC_COLS = H  # timing probe: set to 1 to keep DMA but skip DVE work
_N_PE_TILES = 0   # row-tiles (of 16) computed on PE+ACT instead of DVE
_H_RINGS = 2      # DMA rings for h chunks: 2 = sync+scalar, 3 = +gpsimd


def _build_nc():
    import concourse.bacc as bacc
    import concourse.tile as tile
    import concourse.mybir as mybir

    f32 = mybir.dt.float32
    Alu = mybir.AluOpType
    Act = mybir.ActivationFunctionType

    nc = bacc.Bacc("TRN2", target_bir_lowering=False, debug=False,
                   num_devices=N_CORES)

    h = nc.dram_tensor("h", [R, H], f32, kind="ExternalInput")
    wb = nc.dram_tensor("wb", [128, H], f32, kind="ExternalInput")
    sel2 = nc.dram_tensor("sel2", [2, 256], f32, kind="ExternalInput")
    ones = nc.dram_tensor("ones", [128, 128], f32, kind="ExternalInput")
    ident = nc.dram_tensor("ident", [128, 128], f32, kind="ExternalInput")
    piv0 = nc.dram_tensor("piv0", [128, 1], f32, kind="ExternalInput")
    dcol = nc.dram_tensor("dcol", [128, N_LEVELS], f32, kind="ExternalInput")
    kconst = nc.dram_tensor("kconst", [128, 1], f32, kind="ExternalInput")
    wt = nc.dram_tensor("wt", [128, RT], f32, kind="ExternalInput")
    scores_out = nc.dram_tensor("scores_out", [RT, 128], f32,
                                kind="ExternalOutput")
    mask_out = nc.dram_tensor("mask_out", [RT, 128], f32,
                              kind="ExternalOutput")

    with tile.TileContext(nc) as tc:
        with (
            tc.tile_pool(name="consts", bufs=1) as consts,
            tc.tile_pool(name="hp", bufs=max(2, 10 // NA)) as hp,
            tc.tile_pool(name="junkv", bufs=2) as junkvp,
            tc.tile_pool(name="junka", bufs=2) as junkap,
            tc.tile_pool(name="scp", bufs=2) as scp,
            tc.tile_pool(name="flatp", bufs=3) as flatp,
            tc.tile_pool(name="agp", bufs=3) as agp,
            tc.tile_pool(name="bcp", bufs=2) as bcp,
            tc.tile_pool(name="smalls", bufs=2) as smalls,
            tc.tile_pool(name="pivp", bufs=8) as pivp,
            tc.tile_pool(name="pst", bufs=1, space="PSUM") as pstp,
            tc.tile_pool(name="psb", bufs=2, space="PSUM") as psbp,
            tc.tile_pool(name="psj", bufs=1, space="PSUM") as psjp,
            tc.tile_pool(name="pstr", bufs=1, space="PSUM") as pstrp,
            tc.tile_pool(name="psrow", bufs=2, space="PSUM") as psrowp,
            tc.tile_pool(name="hTp", bufs=2) as hTp,
            tc.tile_pool(name="dram", bufs=2, space="DRAM") as dram,
        ):
            w_sb = consts.tile([128, H], f32)
            nc.sync.dma_start(w_sb[:], wb.ap())
            sel2_sb = consts.tile([2, 256], f32)
            nc.sync.dma_start(sel2_sb[:], sel2.ap())
            ones_sb = consts.tile([128, 128], f32)
            nc.sync.dma_start(ones_sb[:], ones.ap())
            id_sb = consts.tile([128, 128], f32)
            nc.sync.dma_start(id_sb[:], ident.ap())
            piv0_sb = consts.tile([128, 1], f32)
            nc.sync.dma_start(piv0_sb[:], piv0.ap())
            dcol_sb = consts.tile([128, N_LEVELS], f32)
            nc.sync.dma_start(dcol_sb[:], dcol.ap())
            k_sb = consts.tile([128, 1], f32)
            nc.sync.dma_start(k_sb[:], kconst.ap())
            wt_sb = consts.tile([128, RT], f32)
            nc.sync.dma_start(wt_sb[:], wt.ap())

            hv = h.ap().rearrange("(n a p) d -> n p a d", a=NA, p=128)

            n_dve = RT - _N_PE_TILES  # row-tiles on the DVE path

            def emit_pe_rowtile(ht_a, flat_sc, col):
                """PE+ACT matvec for one [128, H] row-tile: PE-transpose
                H-blocks to PSUM, ACT-evict, then 16 accumulating
                [128,1]^T @ [128,128] matmuls -> flat scores row."""
                ps_row = psrowp.tile([1, 128], f32, tag="psrow")
                for piece in range(2):
                    ps_tr = pstrp.tile([128, 1024], f32, tag="pstr")
                    for b in range(8):
                        blk = piece * 8 + b
                        nc.tensor.transpose(
                            ps_tr[:, b * 128:(b + 1) * 128],
                            ht_a[:, blk * 128:(blk + 1) * 128], id_sb[:])
                    hT = hTp.tile([128, 1024], f32, tag="hT")
                    nc.scalar.copy(hT[:], ps_tr[:])
                    for b in range(8):
                        blk = piece * 8 + b
                        nc.tensor.matmul(
                            ps_row[:, :], wt_sb[:, col:col + 1],
                            hT[:, b * 128:(b + 1) * 128],
                            start=(blk == 0), stop=(blk == 15),
                            skip_group_check=True)
                nc.scalar.copy(flat_sc[col:col + 1, :], ps_row[:, :])

            def emit_matvec():
                """Phases 1-2: stream h, matvec on DVE (first n_dve
                row-tiles) and PE+ACT (rest), transpose scores."""
                scores_sb = scp.tile([128, RT], f32, tag="sc")
                flat_sc = flatp.tile([RT, 128], f32, tag="flat")
                # h-chunk loads alternate between the two HWDGE rings
                # (qSPDynamicHW / qActDynamicHW) so transfers overlap; a
                # single ring executes its DMAs strictly FIFO.  All small
                # dependent DMAs live on the gpsimd SWDGE ring so they can
                # never head-of-line-block the bulk stream.
                dma_engines = [nc.sync, nc.scalar, nc.gpsimd][:_H_RINGS]
                for i in range(RT // NA):
                    ht = hp.tile([128, NA, H], f32, tag="ht")
                    dma_engines[i % _H_RINGS].dma_start(ht[:], hv[i])
                    for a in range(NA):
                        col = i * NA + a
                        if col >= n_dve:
                            emit_pe_rowtile(ht[:, a, :], flat_sc, col)
                            continue
                        junkv = junkvp.tile([128, H], f32, tag="jv")
                        nc.vector.scalar_tensor_tensor(
                            junkv[:, 0:_MATVEC_COLS],
                            ht[:, a, 0:_MATVEC_COLS], 0.0,
                            w_sb[:, 0:_MATVEC_COLS],
                            op0=Alu.bypass, op1=Alu.mult,
                            accum_out=scores_sb[:, col:col + 1],
                        )
                ps_t = pstp.tile([n_dve, 128], f32, tag="pst")
                nc.tensor.transpose(
                    ps_t[:], scores_sb[:, 0:n_dve], id_sb[:])
                nc.scalar.copy(flat_sc[0:n_dve, :], ps_t[:])
                return {"flat_sc": flat_sc}

            def emit_exchange(st):
                """Phase 3: scores out + pairwise AllGather (gpsimd ring)."""
                flat_sc = st["flat_sc"]
                nc.gpsimd.dma_start(scores_out.ap(), flat_sc[:])
                ag_in = dram.tile([RT, 128], f32, tag="agin")
                ag_out = dram.tile([2, R], f32, tag="agout")
                nc.gpsimd.dma_start(ag_in[:], flat_sc[:])
                nc.gpsimd.collective_compute(
                    "AllGather", Alu.bypass,
                    replica_groups=[[0, 1], [2, 3], [4, 5], [6, 7]],
                    ins=[ag_in.opt()], outs=[ag_out.opt()],
                )
                ag_sb = agp.tile([2, R], f32, tag="ag")
                nc.gpsimd.dma_start(ag_sb[:], ag_out[:])
                st["ag_sb"] = ag_sb

            def emit_search(st):
                """Phases 4-6: broadcast, 3-level search, mask out."""
                ag_sb = st["ag_sb"]
                flat_sc = st["flat_sc"]
                bc_sb = bcp.tile([128, S], f32, tag="bc")
                for j in range(8):
                    ps_b = psbp.tile([128, 512], f32, tag="psb")
                    hh, nn = j // 4, j % 4
                    nc.tensor.matmul(
                        ps_b[:, :],
                        sel2_sb[:, hh * 128:(hh + 1) * 128],
                        ag_sb[:, nn * 512:(nn + 1) * 512],
                    )
                    nc.scalar.copy(
                        bc_sb[:, j * 512:(j + 1) * 512], ps_b[:])

                piv = pivp.tile([128, 1], f32, tag="piv")
                nc.scalar.copy(piv[:], piv0_sb[:])
                for lvl in range(N_LEVELS):
                    junkv = junkvp.tile([128, H], f32, tag="jv")
                    cnt = smalls.tile([128, 1], f32, tag=f"cnt{lvl}")
                    nc.vector.tensor_scalar(
                        junkv[:, 0:S - N_ACT], bc_sb[:, 0:S - N_ACT],
                        piv[:, 0:1], None,
                        op0=Alu.is_ge, op1=Alu.add, accum_out=cnt[:],
                    )
                    junka = junkap.tile([128, N_ACT], f32, tag="ja")
                    sgn = smalls.tile([128, 1], f32, tag=f"sgn{lvl}")
                    nc.scalar.activation(
                        junka[:], bc_sb[:, S - N_ACT:S], Act.Sign,
                        bias=piv[:, 0:1], scale=-1.0, accum_out=sgn[:],
                    )
                    # cond_pm = sign(2*cnt_dve - sgn - (2K - N_ACT) + 0.5)
                    # in {-1, +1}; argument is a half-integer, never 0.
                    t1 = smalls.tile([128, 1], f32, tag=f"t1{lvl}")
                    nc.vector.scalar_tensor_tensor(
                        t1[:], cnt[:], 2.0, sgn[:],
                        op0=Alu.mult, op1=Alu.subtract,
                    )
                    cond = smalls.tile([128, 1], f32, tag=f"cond{lvl}")
                    nc.scalar.activation(
                        cond[:], t1[:], Act.Sign, bias=k_sb[:, 0:1])
                    # js = sum_p cond_p = 2*j - 128, replicated on all
                    # partitions; piv' = c_l*j + piv + d_l
                    #            = (c_l/2)*js + piv + (d_l + 64*c_l)
                    # (the 64*c_l is folded into dcol on the host)
                    ps_j = psjp.tile([128, 1], f32, tag="psj")
                    nc.tensor.matmul(ps_j[:], ones_sb[:], cond[:])
                    e = smalls.tile([128, 1], f32, tag=f"e{lvl}")
                    nc.vector.tensor_tensor(
                        e[:], piv[:], dcol_sb[:, lvl:lvl + 1], op=Alu.add)
                    piv_n = pivp.tile([128, 1], f32, tag="piv")
                    nc.scalar.activation(
                        piv_n[:], ps_j[:, 0:1], Act.Identity,
                        bias=e[:, 0:1], scale=float(CS[lvl] / 2.0))
                    piv = piv_n

                mask_sb = flatp.tile([RT, 128], f32, tag="mask")
                nc.vector.tensor_scalar(
                    mask_sb[:], flat_sc[:], piv[0:RT, 0:1], None,
                    op0=Alu.is_ge)
                nc.gpsimd.dma_start(mask_out.ap(), mask_sb[:])

            # Software pipeline with a 1-rep skew: emit rep k's matvec +
            # exchange, but rep k-1's search, so no engine's in-order
            # instruction stream ever waits on the current rep's collective
            # before starting the next rep's bulk work.
            prev = None
            for rep in range(_REPS):
                st = emit_matvec()
                if prev is not None:
                    emit_search(prev)
                emit_exchange(st)
                prev = st
            emit_search(prev)

    nc.compile()
    return nc


def _host_inputs(hidden_states, gate_w):
    flat = np.ascontiguousarray(
        np.asarray(hidden_states, dtype=np.float32).reshape(B * S, H))
    wb = np.ascontiguousarray(
        np.broadcast_to(np.asarray(gate_w, dtype=np.float32).reshape(1, H),
                        (128, H)))
    sel2 = np.zeros((2, 256), np.float32)
    sel2[0, :128] = 1.0
    sel2[1, 128:] = 1.0
    ones = np.ones((128, 128), np.float32)
    ident = np.eye(128, dtype=np.float32)
    p = np.arange(128, dtype=np.float32)
    cs = [np.float32(c) for c in CS]
    piv0 = (np.float32(-W0 / 2.0) + p * cs[0]).reshape(128, 1)
    dcol = np.stack(
        [p * (cs[l + 1] - cs[l]) - cs[l] + 64.0 * cs[l]
         for l in range(N_LEVELS)],
        axis=1).astype(np.float32)
    kconst = np.full((128, 1), 0.5 - COMBINE_K, np.float32)
    wtm = np.ascontiguousarray(
        np.asarray(gate_w, dtype=np.float32).reshape(RT, 128).T)

    in_maps = []
    for c in range(N_CORES):
        in_maps.append({
            "h": flat[c * R:(c + 1) * R],
            "wb": wb,
            "sel2": sel2,
            "ones": ones,
            "ident": ident,
            "piv0": piv0,
            "dcol": dcol,
            "kconst": kconst,
            "wt": wtm,
        })
    return in_maps


def _assemble(results):
    scores = np.concatenate(
        [results[c]["scores_out"].reshape(R) for c in range(N_CORES)]
    ).reshape(B, S)
    mask = np.concatenate(
        [results[c]["mask_out"].reshape(R) for c in range(N_CORES)]
    ).reshape(B, S)
    return mask, scores


def get_nc():
    if "nc" not in _CACHE:
        _CACHE["nc"] = _build_nc()
    return _CACHE["nc"]


def kernel(hidden_states, gate_w):
    from concourse.bass_utils import run_bass_kernel_spmd

    nc = get_nc()
    in_maps = _host_inputs(hidden_states, gate_w)
    res = run_bass_kernel_spmd(nc, in_maps, core_ids=list(range(N_CORES)))
    return _assemble(res.results)
